# revision 60
# baseline (speedup 1.0000x reference)
"""DeltaModel Trainium2 kernel (table-gather + fused-solve design).

Pipeline per core (2 batch elements per core, 8 cores data-parallel):

Since embed->FFN->LayerNorm is a pure per-token function of the vocab index
and V=64, compute a 64-row table of LayerNormed keys (and the beta-scaled
-Kb table) ONCE, then per 128-token chunk gather K|(-Kb) token-major AND
feature-major via two one-hot matmuls.  The chunked delta-rule solve keeps
the baseline's proven 2-level structure (block-diag Jacobi + exact
block-Horner) but with only M_JAC=4 Jacobi iterations, with every
elementwise subtract either fused into PSUM matmul accumulation (extra
identity matmul + any-engine copy) or expressed as a single TT-add against
a negated-mask product, spread across DVE/Act/Pool.

Sign bookkeeping: we store -Kb instead of Kb.  Then
  s_ps = K(-Kb)^T           -> strict-upper 32-block mask   = -A_bd^T (negS_bd)
  a_ps = (-Kb)K^T           -> strict-lower off-block mask  = -A_off
  joint Jacobi solves (I+A_bd) X = [K | -Kb | -A_off]:
      X_{j+1} = R + negS_bd^T X_j   (TT add against psum, or fused via I*R)
  -> X = [W0 | -Z0 | -N], transpose of third block = -N^T directly
  Horner: V_{i+1} = Y + (-N^T)^T V_i  (same add/fuse forms)
  -> V = [W | -Z];  negGT = (-Z)^T K;  mt' = F + mt + negGT^T mt  (one psum)
"""

import numpy as np

H = 64
V = 64
B = 16
L = 2048
NCORES = 8
BPC = B // NCORES          # batch per core = 2
NT = 16                    # chunks of 128 tokens per batch element
C = 128                    # chunk length
M_JAC = 4                  # Jacobi iterations (validated vs oracle: ~6e-3)
N_HORNER = 3               # exact for 4x32 block structure
PKW = 708
LN_EPS = 1e-5
D_EPS = 1e-6

# Per-iteration implementation plan: ('sub'|'fused', engine)
#   'sub'   = 1 matmul (negS_bd @ X) + TT-add(R, psum) on 'v' (DVE) or 'g' (Pool)
#   'fused' = 2 matmuls (I@R + negS_bd@X) + plain copy on 'v'/'s'(Act)/'g'
JAC_PLAN = [('sub', 'v'), ('sub', 'g'), ('fused', 's'), ('sub', 'g')]
HORNER_PLAN = [('fused', 's'), ('sub', 'v'), ('sub', 'v')]

_CACHE = {}


def _build_nc(legalize=True):
    import concourse.bass as bass
    import concourse.mybir as mybir
    import concourse.tile as tile
    from concourse import masks

    dt = mybir.dt
    f32 = dt.float32
    bf16 = dt.bfloat16
    i32 = dt.int32
    Alu = mybir.AluOpType
    Act = mybir.ActivationFunctionType
    Axis = mybir.AxisListType

    nc = bass.Bass()

    # seq one-hot encoded on the host: [64, L] f32 per batch element
    seq_p = nc.declare_dram_parameter("seq", [BPC, 64, L], f32, isOutput=False)
    pk_p = nc.declare_dram_parameter("pk", [128, PKW], f32, isOutput=False)
    out_p = nc.declare_dram_parameter("out", [BPC, V], f32, isOutput=True)

    from contextlib import ExitStack
    with tile.TileContext(nc) as tc, ExitStack() as est:
        persist = est.enter_context(tc.tile_pool(name="persist", bufs=1))
        _tcount = [0]
        def _tile(shape, dtype, name=None):
            n = name or f"t{_tcount[0]}"
            _tcount[0] += 1
            return persist.tile(shape, dtype, name=n, tag=n)

        # ---------- constants ----------
        I128r = _tile([128, 128], f32)
        masks.make_identity(nc, I128r[:])
        I128b = _tile([128, 128], bf16)
        nc.vector.tensor_copy(I128b[:], I128r[:])
        I64f = _tile([64, 64], f32)
        nc.vector.tensor_copy(I64f[:], I128r[0:64, 0:64])
        I64b = _tile([64, 64], bf16)
        nc.vector.tensor_copy(I64b[:], I128r[0:64, 0:64])

        ones1x64r = _tile([1, 64], f32)
        nc.gpsimd.memset(ones1x64r[:], 1.0)
        ones1x64 = _tile([1, 64], f32)
        nc.vector.tensor_copy(ones1x64[:], ones1x64r[:])
        ones1x64b = _tile([1, 64], bf16)
        nc.vector.tensor_copy(ones1x64b[:], ones1x64r[:])
        one11r = _tile([1, 1], f32)
        nc.gpsimd.memset(one11r[:], 1.0)
        one11 = _tile([1, 1], f32)
        nc.vector.tensor_copy(one11[:], one11r[:])

        iota_i = _tile([64, 1], i32)
        nc.gpsimd.iota(iota_i[:], pattern=[[0, 1]], base=0, channel_multiplier=1)
        iota_f = _tile([64, 1], f32)
        nc.vector.tensor_copy(iota_f[:], iota_i[:])

        epsc = _tile([64, 1], f32)
        nc.gpsimd.memset(epsc[:], LN_EPS)

        # block-diag strict-upper mask, value -1 (keep S[s,t] with s<t, same
        # 32-block -> gives negS_bd when multiplied with s_ps = K(-Kb)^T...
        # note s_ps is already negated so use +1.  See sign notes in header:
        # s_ps = K @ (-Kb)^T = -(K Kb^T); masked strict-upper-in-block gives
        # -A_bd^T = negS_bd directly with a +1 mask.
        mask_bdsu = _tile([128, 128], f32)
        nc.gpsimd.memset(mask_bdsu[:], 0.0)
        for blk in range(4):
            sub = mask_bdsu[32 * blk:32 * blk + 32, 32 * blk:32 * blk + 32]
            nc.gpsimd.affine_select(
                out=sub, in_=sub, compare_op=Alu.is_ge, fill=1.0,
                base=0, pattern=[[-1, 32]], channel_multiplier=1)

        # block-diag strict-LOWER mask (keep A_bd's transpose orientation):
        # a_ps = (-Kb)K^T = -A-low, so masking in-block strict-lower gives
        # -A_bd (low) = negAbd, used as lhsT for S2 = S_bd^2.
        mask_bdsl = _tile([128, 128], f32)
        nc.gpsimd.memset(mask_bdsl[:], 0.0)
        for blk in range(4):
            sub = mask_bdsl[32 * blk:32 * blk + 32, 32 * blk:32 * blk + 32]
            nc.gpsimd.affine_select(
                out=sub, in_=sub, compare_op=Alu.is_ge, fill=1.0,
                base=0, pattern=[[1, 32]], channel_multiplier=-1)

        # off-block strict-lower mask (+1): a_ps = (-Kb) K^T = -A-low, so the
        # masked product is -A_off (which is what the solve wants as RHS).
        mask_offsl = _tile([128, 128], f32)
        nc.gpsimd.memset(mask_offsl[:], 1.0)
        nc.gpsimd.affine_select(
            out=mask_offsl[:], in_=mask_offsl[:], compare_op=Alu.is_gt,
            fill=0.0, base=0, pattern=[[-1, 128]], channel_multiplier=1)
        for blk in range(4):
            nc.gpsimd.memset(
                mask_offsl[32 * blk:32 * blk + 32, 32 * blk:32 * blk + 32], 0.0)

        # pair masks: two copies side by side for [C,256] paired products
        mask_bdsu2 = _tile([128, 256], f32)
        nc.gpsimd.memset(mask_bdsu2[:], 0.0)
        for half in range(2):
            for blk in range(4):
                sub = mask_bdsu2[32 * blk:32 * blk + 32,
                                 half * 128 + 32 * blk:half * 128 + 32 * blk + 32]
                nc.gpsimd.affine_select(
                    out=sub, in_=sub, compare_op=Alu.is_ge, fill=1.0,
                    base=0, pattern=[[-1, 32]], channel_multiplier=1)
        mask_bdsl2 = _tile([128, 256], f32)
        nc.gpsimd.memset(mask_bdsl2[:], 0.0)
        for half in range(2):
            for blk in range(4):
                sub = mask_bdsl2[32 * blk:32 * blk + 32,
                                 half * 128 + 32 * blk:half * 128 + 32 * blk + 32]
                nc.gpsimd.affine_select(
                    out=sub, in_=sub, compare_op=Alu.is_ge, fill=1.0,
                    base=0, pattern=[[1, 32]], channel_multiplier=-1)
        mask_offsl2 = _tile([128, 256], f32)
        nc.gpsimd.memset(mask_offsl2[:], 1.0)
        for half in range(2):
            sub = mask_offsl2[:, half * 128:half * 128 + 128]
            nc.gpsimd.affine_select(
                out=sub, in_=sub, compare_op=Alu.is_gt,
                fill=0.0, base=0, pattern=[[-1, 128]], channel_multiplier=1)
            for blk in range(4):
                nc.gpsimd.memset(
                    mask_offsl2[32 * blk:32 * blk + 32,
                                half * 128 + 32 * blk:half * 128 + 32 * blk + 32], 0.0)

        # ---------- parameters via one packed DMA ----------
        pk_sb = _tile([128, PKW], f32, name="pk_sb")
        nc.sync.dma_start(pk_sb[:], pk_p[:])
        W2 = pk_sb[:, 0:64]            # [128(2H), 64]
        W1 = pk_sb[0:64, 64:192]       # [64, 128]
        embT = pk_sb[0:64, 192:256]    # [64(feat), 64(vocab)] = embed^T
        Wr = pk_sb[0:64, 256:320]
        Wo = pk_sb[0:64, 320:384]
        b1c = pk_sb[:, 384:385]
        gar = pk_sb[0:1, 385:449]
        ber = pk_sb[0:1, 449:513]
        b2r = pk_sb[0:1, 513:577]
        brr = pk_sb[0:1, 577:641]
        bor = pk_sb[0:1, 641:705]

        # seq DMA: one contiguous [1, L] bf16 row per batch element
        ohf = []
        ohb = []
        for b in range(BPC):
            of_ = _tile([64, L], f32, name=f"ohf{b}")
            for s4 in range(4):
                seg = L // 4
                nc.sync.dma_start(of_[:, s4 * seg:(s4 + 1) * seg],
                                  seq_p[b, :, s4 * seg:(s4 + 1) * seg])
            ob_ = _tile([64, L], bf16, name=f"ohb{b}")
            for s8 in range(8):
                seg = L // 8
                nc.gpsimd.tensor_copy(ob_[:, s8 * seg:(s8 + 1) * seg],
                                      of_[:, s8 * seg:(s8 + 1) * seg])
            ohf.append(of_)
            ohb.append(ob_)

        # psum pools (bank granular, 8 banks total), batch-paired layout:
        #   kt [64,512] x1 | r [C,256] x1 | sa [C,512] x2 | sj [C,512] x3 | sm x1
        pp_kt = est.enter_context(tc.tile_pool(name="pp_kt", bufs=1, space="PSUM"))
        pp_r = est.enter_context(tc.tile_pool(name="pp_r", bufs=1, space="PSUM"))
        pp_sa = est.enter_context(tc.tile_pool(name="pp_sa", bufs=1, space="PSUM"))
        pp_sj = est.enter_context(tc.tile_pool(name="pp_sj", bufs=5, space="PSUM"))

        # sbuf pools (SBUF is plentiful here - tiles are small)
        sb_oh = est.enter_context(tc.tile_pool(name="sb_oh", bufs=6))
        sb_r = est.enter_context(tc.tile_pool(name="sb_r", bufs=1))
        sb_kt = est.enter_context(tc.tile_pool(name="sb_kt", bufs=6))
        sb_sbd = est.enter_context(tc.tile_pool(name="sb_sbd", bufs=1))
        sb_x = est.enter_context(tc.tile_pool(name="sb_x", bufs=10))
        sb_nt = est.enter_context(tc.tile_pool(name="sb_nt", bufs=4))
        sb_v = est.enter_context(tc.tile_pool(name="sb_v", bufs=8))
        sb_mt = est.enter_context(tc.tile_pool(name="sb_mt", bufs=6))
        sb_small = est.enter_context(tc.tile_pool(name="sb_small", bufs=6))

        # ---------- LN'd key table (64 rows, once) ----------
        tbl_ps = pp_sa.tile([128, 512], f32, name="tbl_ps", tag="psa")
        # g1 feature-major: [128(2H), 64(vocab)] = relu(W1^T embT + b1)
        g_ps = tbl_ps[:, 0:64]
        nc.tensor.matmul(g_ps, lhsT=W1, rhs=embT, start=True, stop=True)
        g1f = _tile([128, 64], f32, name="g1f")
        nc.vector.tensor_scalar(out=g1f[:], in0=g_ps, scalar1=b1c,
                                scalar2=0.0, op0=Alu.add, op1=Alu.max)
        # x vocab-major [64(vocab), 64(feat)] = g1^T W2 + embed + b2
        x_ps = tbl_ps[0:64, 64:128]
        nc.tensor.matmul(x_ps, lhsT=g1f[:], rhs=W2, start=True, stop=False)
        nc.tensor.matmul(x_ps, lhsT=embT, rhs=I64f[:], start=False, stop=False)
        nc.tensor.matmul(x_ps, lhsT=ones1x64[:], rhs=b2r, start=False, stop=True)
        # LayerNorm over feature dim (free axis)
        s1 = _tile([64, 1], f32)
        nc.vector.tensor_reduce(s1[:], x_ps, axis=Axis.X, op=Alu.add)
        mu = _tile([64, 1], f32)
        nc.vector.tensor_scalar_mul(mu[:], s1[:], 1.0 / H)
        xc = _tile([64, 64], f32)
        nc.vector.tensor_scalar(out=xc[:], in0=x_ps, scalar1=mu[:],
                                scalar2=None, op0=Alu.subtract)
        sqs = _tile([64, 64], f32)
        ssq = _tile([64, 1], f32)
        nc.scalar.activation(sqs[:], xc[:], Act.Square, accum_out=ssq[:])
        sroot = _tile([64, 1], f32)
        nc.scalar.activation(sroot[:], ssq[:], Act.Sqrt,
                             bias=epsc[:], scale=1.0 / H)
        rstd = _tile([64, 1], f32)
        nc.vector.reciprocal(rstd[:], sroot[:])
        kk = _tile([64, 64], f32)
        nc.vector.tensor_scalar(out=kk[:], in0=xc[:], scalar1=rstd[:],
                                scalar2=None, op0=Alu.mult)
        gb_ps = tbl_ps[0:64, 128:192]
        nc.tensor.matmul(gb_ps, lhsT=ones1x64[:], rhs=gar, start=True, stop=True)
        gamma_bc = _tile([64, H], f32)
        nc.vector.tensor_copy(gamma_bc[:], gb_ps)
        bb_ps = tbl_ps[0:64, 192:256]
        nc.tensor.matmul(bb_ps, lhsT=ones1x64[:], rhs=ber, start=True, stop=True)
        beta_bc = _tile([64, H], f32)
        nc.vector.tensor_copy(beta_bc[:], bb_ps)
        kg = _tile([64, 64], f32)
        nc.vector.tensor_mul(kg[:], kk[:], gamma_bc[:])
        # table tile: cols 0:64 = LN'd keys, cols 64:128 = -beta * keys
        tab = _tile([64, 128], bf16, name="tab")
        nc.vector.tensor_add(tab[:, 0:64], kg[:], beta_bc[:])
        sqk = _tile([64, 64], f32)
        ssk = _tile([64, 1], f32)
        nc.scalar.activation(sqk[:], tab[:, 0:64], Act.Square, accum_out=ssk[:])
        negbtv = _tile([64, 1], f32)
        nc.vector.tensor_scalar(out=negbtv[:], in0=ssk[:], scalar1=D_EPS,
                                scalar2=-1.0, op0=Alu.add, op1=Alu.mult)
        negbeta = _tile([64, 1], f32)
        nc.vector.reciprocal(negbeta[:], negbtv[:])
        nc.vector.tensor_scalar(out=tab[:, 64:128], in0=tab[:, 0:64],
                                scalar1=negbeta[:], scalar2=None, op0=Alu.mult)

        qT = [_tile([64, 1], bf16, name=f"qT{b}") for b in range(BPC)]
        mt_cur = [None] * BPC

        eng = {'v': nc.vector, 'g': nc.gpsimd}

        # ---------- phase 1: paired gather fronts (both batch elements) ----
        # Rp layout per chunk-pair: [KK_b0 | KK_b1 | -Aoff_b0 | -Aoff_b1]
        Rp_all = [None] * NT
        negS_all = [None] * NT
        S2_all = [None] * NT
        for c in range(NT):
            OHs = [ohb[b][:, c * C:(c + 1) * C] for b in range(BPC)]
            if c == NT - 1:
                for b in range(BPC):
                    qt_ps = pp_r.tile([64, 1], f32, name="qt_ps", tag="pr")
                    nc.tensor.matmul(qt_ps[:], lhsT=tab[:, 0:64],
                                     rhs=OHs[b][:, 127:128], start=True, stop=True)
                    nc.vector.tensor_copy(qT[b][:], qt_ps[:])
                    nc.gpsimd.affine_select(
                        out=OHs[b], in_=OHs[b], compare_op=Alu.is_ge, fill=0.0,
                        base=126, pattern=[[-1, C]], channel_multiplier=0)

            rp = pp_r.tile([C, 256], f32, name="rp", tag="pr")
            nc.tensor.matmul(rp[:, 0:128], lhsT=OHs[0], rhs=tab[:],
                             start=True, stop=True)
            nc.tensor.matmul(rp[:, 128:256], lhsT=OHs[1], rhs=tab[:],
                             start=True, stop=True)
            Rp = sb_r.tile([C, 512], bf16, name=f"Rp{c}", tag=f"Rp{c}")
            nc.scalar.copy(Rp[:, 0:256], rp[:])

            ktp = pp_kt.tile([64, 512], f32, name="ktp", tag="pkt")
            nc.tensor.matmul(ktp[:, 0:128], lhsT=tab[:, 0:64], rhs=OHs[0],
                             start=True, stop=True)
            nc.tensor.matmul(ktp[:, 128:256], lhsT=tab[:, 64:128], rhs=OHs[0],
                             start=True, stop=True)
            nc.tensor.matmul(ktp[:, 256:384], lhsT=tab[:, 0:64], rhs=OHs[1],
                             start=True, stop=True)
            nc.tensor.matmul(ktp[:, 384:512], lhsT=tab[:, 64:128], rhs=OHs[1],
                             start=True, stop=True)
            KTp = sb_kt.tile([64, 512], bf16, name="KTp")
            nc.scalar.copy(KTp[:], ktp[:])

            sap = pp_sa.tile([C, 512], f32, name="sap", tag="psa")
            nc.tensor.matmul(sap[:, 0:128], lhsT=KTp[:, 0:128],
                             rhs=KTp[:, 128:256], start=True, stop=True)
            nc.tensor.matmul(sap[:, 128:256], lhsT=KTp[:, 256:384],
                             rhs=KTp[:, 384:512], start=True, stop=True)
            nc.tensor.matmul(sap[:, 256:384], lhsT=KTp[:, 128:256],
                             rhs=KTp[:, 0:128], start=True, stop=True)
            nc.tensor.matmul(sap[:, 384:512], lhsT=KTp[:, 384:512],
                             rhs=KTp[:, 256:384], start=True, stop=True)
            sasb = sb_kt.tile([C, 512], bf16, name="sasb")
            nc.vector.tensor_copy(sasb[:], sap[:])
            negS = sb_sbd.tile([C, 256], bf16, name=f"nS{c}", tag=f"nS{c}")
            nc.gpsimd.tensor_mul(negS[:], sasb[:, 0:256], mask_bdsu2[:])
            nc.gpsimd.tensor_mul(Rp[:, 256:512], sasb[:, 256:512], mask_offsl2[:])
            negAbd2 = sb_kt.tile([C, 256], bf16, name="negAbd2")
            nc.gpsimd.tensor_mul(negAbd2[:], sasb[:, 256:512], mask_bdsl2[:])
            # S2 pair, psums reuse the s-pair region (its reader is done)
            nc.tensor.matmul(sap[:, 0:128], lhsT=negAbd2[:, 0:128],
                             rhs=negS[:, 0:128], start=True, stop=True)
            nc.tensor.matmul(sap[:, 128:256], lhsT=negAbd2[:, 128:256],
                             rhs=negS[:, 128:256], start=True, stop=True)
            S2p = sb_sbd.tile([C, 256], bf16, name=f"S2_{c}", tag=f"S2_{c}")
            nc.vector.tensor_copy(S2p[:], sap[:, 0:256])
            Rp_all[c] = Rp
            negS_all[c] = negS
            S2_all[c] = S2p

        # ---------- phase 2: paired solves + per-b state chain ----------
        for c in range(NT):
            Rp = Rp_all[c]
            negS = negS_all[c]
            S2p = S2_all[c]
            # v = Rp - A_bd Rp   (deg-1, all 512 cols)
            sjv = pp_sj.tile([C, 512], f32, name="sjv", tag="psj")
            nc.tensor.matmul(sjv[:, 0:128], lhsT=negS[:, 0:128],
                             rhs=Rp[:, 0:128], start=True, stop=True)
            nc.tensor.matmul(sjv[:, 128:256], lhsT=negS[:, 128:256],
                             rhs=Rp[:, 128:256], start=True, stop=True)
            nc.tensor.matmul(sjv[:, 256:384], lhsT=negS[:, 0:128],
                             rhs=Rp[:, 256:384], start=True, stop=True)
            nc.tensor.matmul(sjv[:, 384:512], lhsT=negS[:, 128:256],
                             rhs=Rp[:, 384:512], start=True, stop=True)
            v = sb_x.tile([C, 512], bf16, name="v")
            nc.vector.tensor_add(v[:], Rp[:], sjv[:])
            # w = v + A^2 v      (deg-3)
            sjw = pp_sj.tile([C, 512], f32, name="sjw", tag="psj")
            nc.tensor.matmul(sjw[:, 0:128], lhsT=S2p[:, 0:128],
                             rhs=v[:, 0:128], start=True, stop=True)
            nc.tensor.matmul(sjw[:, 128:256], lhsT=S2p[:, 128:256],
                             rhs=v[:, 128:256], start=True, stop=True)
            nc.tensor.matmul(sjw[:, 256:384], lhsT=S2p[:, 0:128],
                             rhs=v[:, 256:384], start=True, stop=True)
            nc.tensor.matmul(sjw[:, 384:512], lhsT=S2p[:, 128:256],
                             rhs=v[:, 384:512], start=True, stop=True)
            wst = sb_x.tile([C, 512], bf16, name="wst")
            nc.scalar.copy(wst[:], sjw[:])
            w = sb_x.tile([C, 512], bf16, name="w")
            nc.gpsimd.tensor_add(w[:], v[:], wst[:])
            # X5 = v + A^2 w on the KK columns only (deg-5 keys, deg-3 N)
            nc.tensor.matmul(sjv[:, 0:128], lhsT=S2p[:, 0:128],
                             rhs=w[:, 0:128], start=True, stop=True)
            nc.tensor.matmul(sjv[:, 128:256], lhsT=S2p[:, 128:256],
                             rhs=w[:, 128:256], start=True, stop=True)
            X5 = sb_x.tile([C, 256], bf16, name="X5")
            nc.vector.tensor_add(X5[:], v[:, 0:256], sjv[:, 0:256])
            # -N^T via XBAR DMA transpose (both halves)
            negNT = sb_nt.tile([C, 256], bf16, name="negNT")
            nc.sync.dma_start_transpose(negNT[:, 0:128], w[:, 256:384])
            nc.sync.dma_start_transpose(negNT[:, 128:256], w[:, 384:512])
            # Horner: V' = Y + (-N^T)^T V, three rounds
            Y = X5[:]
            # h1 (fused, Act copy): psum in sjv[:, 256:512]
            nc.tensor.matmul(sjv[:, 256:384], lhsT=I128b[:], rhs=Y[:, 0:128],
                             start=True, stop=False)
            nc.tensor.matmul(sjv[:, 256:384], lhsT=negNT[:, 0:128],
                             rhs=Y[:, 0:128], start=False, stop=True)
            nc.tensor.matmul(sjv[:, 384:512], lhsT=I128b[:], rhs=Y[:, 128:256],
                             start=True, stop=False)
            nc.tensor.matmul(sjv[:, 384:512], lhsT=negNT[:, 128:256],
                             rhs=Y[:, 128:256], start=False, stop=True)
            V1 = sb_v.tile([C, 256], bf16, name="V1")
            nc.scalar.copy(V1[:], sjv[:, 256:512])
            # h2 (sub, DVE): psum in sjw[:, 0:256]
            nc.tensor.matmul(sjw[:, 0:128], lhsT=negNT[:, 0:128],
                             rhs=V1[:, 0:128], start=True, stop=True)
            nc.tensor.matmul(sjw[:, 128:256], lhsT=negNT[:, 128:256],
                             rhs=V1[:, 128:256], start=True, stop=True)
            V2 = sb_v.tile([C, 256], bf16, name="V2")
            nc.vector.tensor_add(V2[:], Y, sjw[:, 0:256])
            # h3 (sub, DVE): psum in sjw[:, 256:512]
            nc.tensor.matmul(sjw[:, 256:384], lhsT=negNT[:, 0:128],
                             rhs=V2[:, 0:128], start=True, stop=True)
            nc.tensor.matmul(sjw[:, 384:512], lhsT=negNT[:, 128:256],
                             rhs=V2[:, 128:256], start=True, stop=True)
            V3 = sb_v.tile([C, 256], bf16, name="V3")
            nc.vector.tensor_add(V3[:], Y, sjw[:, 256:512])

            # state update: ngt psum+copy paired (off the serial mt chain),
            # mt updates kept separate per b (chains stay decoupled)
            ngt_ps = pp_r.tile([H, 2 * H], f32, name="ngt_ps", tag="pr")
            for b in range(BPC):
                o = 128 * b
                nc.tensor.matmul(ngt_ps[:, b * H:(b + 1) * H],
                                 lhsT=V3[:, o + 64:o + 128],
                                 rhs=Rp[:, o:o + 64], start=True, stop=True)
            negGTp = sb_mt.tile([H, 2 * H], bf16, name="negGTp")
            nc.scalar.copy(negGTp[:], ngt_ps[:])
            for b in range(BPC):
                o = 128 * b
                negGT = negGTp[:, b * H:(b + 1) * H]
                mt_ps = pp_r.tile([H, H], f32, name="mt_ps", tag="pr")
                if c == 0:
                    nc.tensor.matmul(mt_ps[:], lhsT=Rp[:, o:o + 64],
                                     rhs=V3[:, o:o + 64], start=True, stop=True)
                else:
                    nc.tensor.matmul(mt_ps[:], lhsT=Rp[:, o:o + 64],
                                     rhs=V3[:, o:o + 64], start=True, stop=False)
                    nc.tensor.matmul(mt_ps[:], lhsT=I64b[:], rhs=mt_cur[b][:],
                                     start=False, stop=False)
                    nc.tensor.matmul(mt_ps[:], lhsT=negGT, rhs=mt_cur[b][:],
                                     start=False, stop=True)
                mt_new = sb_mt.tile([H, H], bf16, name="mt_new")
                nc.scalar.copy(mt_new[:], mt_ps[:])
                mt_cur[b] = mt_new

        # ---------- readout head (paired) ----------
        cx_ps = pp_r.tile([H, 2], f32, name="cx_ps", tag="pr")
        for b in range(BPC):
            nc.tensor.matmul(cx_ps[:, b:b + 1], lhsT=mt_cur[b][:], rhs=qT[b][:],
                             start=True, stop=True)
        ctx = sb_small.tile([H, 2], f32, name="ctx")
        nc.vector.tensor_copy(ctx[:], cx_ps[:])
        ones1x2 = sb_small.tile([1, 2], f32, name="ones1x2")
        nc.gpsimd.memset(ones1x2[:], 1.0)
        z_ps = pp_r.tile([H, 2], f32, name="z_ps", tag="pr")
        nc.tensor.matmul(z_ps[:], lhsT=Wr, rhs=ctx[:], start=True, stop=False)
        nc.tensor.matmul(z_ps[:], lhsT=brr, rhs=ones1x2[:],
                         start=False, stop=True)
        zt = sb_small.tile([H, 2], f32, name="zt")
        nc.vector.tensor_copy(zt[:], z_ps[:])
        y_ps = pp_r.tile([V, 2], f32, name="y_ps", tag="pr")
        nc.tensor.matmul(y_ps[:], lhsT=Wo, rhs=zt[:], start=True, stop=False)
        nc.tensor.matmul(y_ps[:], lhsT=bor, rhs=ones1x2[:],
                         start=False, stop=True)
        yt = sb_small.tile([V, 2], f32, name="yt")
        nc.vector.tensor_copy(yt[:], y_ps[:])
        nc.sync.dma_start(out_p[0, :, None], yt[:, 0:1])
        nc.scalar.dma_start(out_p[1, :, None], yt[:, 1:2])

    if legalize:
        _legalize_waits(nc, mybir)
    return nc


def _legalize_waits(nc, mybir):
    """This walrus build encodes at most one sync-wait per instruction.
    Split multi-wait instructions into single-wait NoOp prefixes on the
    same engine (engine queues execute in order, so semantics hold)."""
    k = 0
    for blk in nc.main_func.blocks:
        insts = blk.instructions
        out = []
        changed = False
        for inst in list(insts):
            si = inst.sync_info
            waits = list(si.on_wait) if si is not None and si.on_wait else []
            if len(waits) > 1:
                for w in waits[:-1]:
                    nop = mybir.InstNoOp(name=f"I-wsplit-{k}", ins=[], outs=[])
                    k += 1
                    nop.engine = inst.engine
                    nop.sync_info = mybir.SyncInfo(on_wait=[w], on_update=[])
                    out.append(nop)
                si.on_wait = [waits[-1]]
                changed = True
            out.append(inst)
        if changed:
            while len(insts):
                insts.pop()
            for x in out:
                insts.append(x)


def pack_params(inputs):
    g = lambda k: np.asarray(inputs[k], dtype=np.float32)
    pk = np.zeros((128, PKW), np.float32)
    pk[:, 0:64] = g("W2")
    pk[0:64, 64:192] = g("W1")
    pk[0:64, 192:256] = g("embed").T
    pk[0:64, 256:320] = g("Wr")
    pk[0:64, 320:384] = g("Wo")
    pk[:, 384] = g("b1")
    pk[0, 385:449] = g("gamma")
    pk[0, 449:513] = g("beta")
    pk[0, 513:577] = g("b2")
    pk[0, 577:641] = g("br")
    pk[0, 641:705] = g("bo")
    return np.ascontiguousarray(pk)


def _get_nc():
    if "nc" not in _CACHE:
        _CACHE["nc"] = _build_nc()
    return _CACHE["nc"]


def kernel(**inputs):
    from concourse.bass_utils import run_bass_kernel_spmd

    nc = _get_nc()
    seq = np.asarray(inputs["seq"], dtype=np.int64)
    oh = np.zeros((B, 64, L), np.float32)
    oh[np.arange(B)[:, None], seq, np.arange(L)[None, :]] = 1.0
    seqb = np.ascontiguousarray(oh)
    pk = pack_params(inputs)
    in_maps = []
    for core in range(NCORES):
        m = {"seq": np.ascontiguousarray(seqb[core * BPC:(core + 1) * BPC]),
             "pk": pk}
        in_maps.append(m)
    res = run_bass_kernel_spmd(nc, in_maps, core_ids=list(range(NCORES)))
    out = np.concatenate([r["out"] for r in res.results], axis=0)
    return out.astype(np.float32)


if __name__ == "__main__":
    d = np.load("/root/problem/inputs.npz")
    y = kernel(**{k: d[k] for k in d.files})
    o = np.load("/root/problem/oracle.npz")
    rel = np.abs(y - o["y"]).max() / np.abs(o["y"]).max()
    print("Relative error:", rel)
